# revision 3
# baseline (speedup 1.0000x reference)
"""Trainium2 Bass kernel for Llama-style GQA attention prefill (S=2048), v6.

Sharding: tensor-parallel over heads across 8 NeuronCores.
Each core owns 4 query heads + 1 KV head (GQA group-aligned), computes
its partial o_proj contribution (Wo input-sharded), and the host sums
the 8 fp16 partials in fp32.

v6 changes vs v2:
- projections and attention are software-pipelined: attention for
  q-block qb is emitted immediately after projection seq-block sb=qb
  (causality makes its K/V available), so the Scalar-engine exp work of
  one q-block overlaps the PE projection matmuls of the next.
- PSUM is partitioned so both phases coexist: projection accumulators /
  transpose / RoPE-swap share one 3-slot pool; scores 2; attention
  accumulators 2; denominators 1 (pair shares a bank at rows 0/32).
- accumulator evacuation and the RoPE bf16 staging copy run on the
  Vector engine, keeping the Scalar engine for exp + fp32 staging.
- Wo is DMA'd late (after the x-tile pool frees) to fit SBUF.
- o_proj loops (seq-tile, hid-group, head); PSUM evacuation split
  between Scalar and Vector engines; fp16 output, one 1MB DMA per
  128-row seq tile.
"""

import sys

if "/opt/trn_rl_repo" not in sys.path:
    sys.path.insert(0, "/opt/trn_rl_repo")

import numpy as np
import ml_dtypes

BF = ml_dtypes.bfloat16

S = 2048
HID = 4096
D = 128
H = 32
HKV = 8
NCORES = 8
NQ = H // NCORES  # 4 query heads per core
ROPE_THETA = 10000.0

_NC_CACHE = {}


def build_nc(s=S, hid=HID, nq=NQ, reps=1):
    """Build the per-core Bass program (same program for all 8 cores).

    reps > 1 repeats the whole computation (for timing amplification:
    marginal cost per rep == true device time of one run).
    """
    import concourse.bass as bass
    import concourse.mybir as mybir
    import concourse.tile as tile
    from concourse import bacc
    from concourse.masks import make_identity

    f32 = mybir.dt.float32
    f16 = mybir.dt.float16
    bf16 = mybir.dt.bfloat16
    Exp = mybir.ActivationFunctionType.Exp

    KB = hid // 128   # hidden contraction blocks
    SBn = s // 512    # 512-wide sequence blocks
    KTn = s // 128    # 128-wide key tiles
    STn = s // 128    # 128-wide seq tiles
    NDB = nq + 2      # projection d-blocks: k, v, q0..q{nq-1}
    WC = NDB * 128    # wqkvT columns
    HB = hid // 512   # output hidden blocks

    nc = bacc.Bacc("TRN2")

    xT_d = nc.dram_tensor("xT", [hid, s], bf16, kind="ExternalInput")
    w_d = nc.dram_tensor("wqkvT", [hid, WC], bf16, kind="ExternalInput")
    wo_d = nc.dram_tensor("woT", [nq * 128, hid], bf16, kind="ExternalInput")
    cos2_d = nc.dram_tensor("cos2", [128, s], f32, kind="ExternalInput")
    sin2_d = nc.dram_tensor("sin2", [128, s], f32, kind="ExternalInput")
    tri_d = nc.dram_tensor("tri", [128, 128], bf16, kind="ExternalInput")
    swpm_d = nc.dram_tensor("swpm", [128, 128], bf16, kind="ExternalInput")
    out_d = nc.dram_tensor("out", [s, hid], f16, kind="ExternalOutput")

    with tile.TileContext(nc) as tc:
      for rep in range(reps):
        with (
            tc.tile_pool(name=f"const{rep}", bufs=1) as const_pool,
            tc.tile_pool(name=f"qkv{rep}", bufs=1) as qkv_pool,
        ):
            tri = const_pool.tile([128, 128], bf16, tag="tri")
            ones = const_pool.tile([128, 1], bf16, tag="ones")
            ones1f = const_pool.tile([1, 128], f32, tag="ones1f")
            ident = const_pool.tile([128, 128], bf16, tag="ident")
            swpm = const_pool.tile([128, 128], bf16, tag="swpm")
            nc.sync.dma_start(tri, tri_d[:, :])
            nc.sync.dma_start(swpm, swpm_d[:, :])
            nc.vector.memset(ones, 1.0)
            nc.vector.memset(ones1f, 1.0)
            make_identity(nc, ident)

            # persistent per-head tensors
            qk = [
                qkv_pool.tile([128, s], bf16, tag=f"qk{i}", name=f"qk{i}")
                for i in range(nq + 1)
            ]  # qk[0..nq-1] = q heads (T layout [d, s]); qk[nq] = kT
            vt = qkv_pool.tile([128, KTn, 128], bf16, tag="vt")
            at = [
                qkv_pool.tile([128, s], bf16, tag=f"at{i}", name=f"at{i}")
                for i in range(nq)
            ]  # attnT (unnormalized, then scaled in place) per head [d, s]
            kT = qk[nq]

            # ------- Phases 1+2 interleaved over seq/q blocks -------
            with (
                tc.tile_pool(name=f"csn{rep}", bufs=1) as cs_pool,
                tc.tile_pool(name=f"xt{rep}", bufs=2) as xt_pool,
                tc.tile_pool(name=f"wsb{rep}", bufs=1) as w_pool,
                tc.tile_pool(name=f"pp{rep}", bufs=3, space="PSUM") as pp,
                tc.tile_pool(name=f"scp{rep}", bufs=2, space="PSUM") as scp,
                tc.tile_pool(name=f"atp{rep}", bufs=1, space="PSUM") as atp,
                tc.tile_pool(name=f"dnp{rep}", bufs=1, space="PSUM") as dnp,
                tc.tile_pool(name=f"rtmp{rep}", bufs=3) as rt,
                tc.tile_pool(name=f"vstage{rep}", bufs=2) as vs,
                tc.tile_pool(name=f"exps{rep}", bufs=6) as exps,
                tc.tile_pool(name=f"dsm{rep}", bufs=4) as dsm,
            ):
                cos2 = cs_pool.tile([128, s], f32, tag="cos2")
                sin2 = cs_pool.tile([128, s], f32, tag="sin2")
                nc.sync.dma_start(cos2, cos2_d[:, :])
                nc.sync.dma_start(sin2, sin2_d[:, :])
                w_sb = w_pool.tile([128, KB, WC], bf16, tag="wsb")
                wv_view = w_d[:, :].rearrange("(t p) c -> p t c", p=128)
                for kb in range(KB):
                    nc.sync.dma_start(w_sb[:, kb, :], wv_view[:, kb, :])

                for sb in range(SBn):
                    # ---- projections + RoPE for seq block sb ----
                    sl = slice(sb * 512, (sb + 1) * 512)
                    xts = []
                    for kb in range(KB):
                        xtile = xt_pool.tile(
                            [128, 512], bf16, tag=f"xt{kb}", name=f"xt{kb}"
                        )
                        nc.sync.dma_start(
                            xtile, xT_d[kb * 128 : (kb + 1) * 128, sl]
                        )
                        xts.append(xtile)
                    # d-block order: k(0), v(1), then q heads (2..)
                    for db in range(NDB):
                        ps = pp.tile([128, 512], f32, tag="pp", name="ps")
                        for kb in range(KB):
                            nc.tensor.matmul(
                                ps,
                                w_sb[:, kb, db * 128 : (db + 1) * 128],
                                xts[kb],
                                start=(kb == 0),
                                stop=(kb == KB - 1),
                            )
                        if db == 1:
                            # v: cast to bf16 then transpose to natural [s, d]
                            vstg = vs.tile([128, 512], bf16, tag="vstg")
                            nc.scalar.copy(vstg, ps)
                            for j in range(4):
                                tps = pp.tile(
                                    [128, 128], bf16, tag="pp", name="tps"
                                )
                                nc.tensor.transpose(
                                    tps, vstg[:, j * 128 : (j + 1) * 128], ident
                                )
                                nc.scalar.copy(vt[:, sb * 4 + j, :], tps)
                        else:
                            # RoPE: dst = p * COS2 + swap(p) * SIN2
                            # (half-swap via bf16 PE permutation matmul)
                            dst = qk[nq] if db == 0 else qk[db - 2]
                            pcp = rt.tile([128, 512], f32, tag="pcp")
                            nc.scalar.copy(pcp, ps)
                            pcb = vs.tile([128, 512], bf16, tag="pcb")
                            nc.vector.tensor_copy(pcb, ps)
                            sps = pp.tile(
                                [128, 512], f32, tag="pp", name="sps"
                            )
                            nc.tensor.matmul(
                                sps, swpm, pcb, start=True, stop=True
                            )
                            nc.vector.tensor_mul(pcp, pcp, cos2[:, sl])
                            swp = rt.tile([128, 512], f32, tag="swp")
                            nc.vector.tensor_mul(swp, sps, sin2[:, sl])
                            nc.vector.tensor_add(dst[:, sl], pcp, swp)

                    # ---- attention for q-block qb == sb ----
                    qb = sb
                    qsl = slice(qb * 512, (qb + 1) * 512)
                    nkt = 4 * qb + 4
                    for hp in range(0, nq, 2):  # head pairs
                        pair = (hp, hp + 1)
                        accs = {
                            h: atp.tile(
                                [128, 512], f32, tag=f"acc{h % 2}",
                                name=f"acc{h % 2}",
                            )
                            for h in pair
                        }
                        dnt = dnp.tile([128, 512], f32, tag="dn", name="dnt")
                        for kt in range(nkt):
                            jstart = max(0, 128 * (kt - 4 * qb))
                            w = 512 - jstart
                            q_lo = qb * 512 + jstart
                            exs = {}
                            for h in pair:
                                sc = scp.tile(
                                    [128, 512], f32, tag="sc", name="sc"
                                )
                                nc.tensor.matmul(
                                    sc[:, :w],
                                    kT[:, kt * 128 : (kt + 1) * 128],
                                    qk[h][:, q_lo : (qb + 1) * 512],
                                    start=True,
                                    stop=True,
                                )
                                ex = exps.tile(
                                    [128, 512], bf16, tag="ex", name="ex"
                                )
                                nc.scalar.activation(ex[:, :w], sc[:, :w], Exp)
                                if kt >= 4 * qb:
                                    nc.vector.tensor_mul(
                                        ex[:, 0:128], ex[:, 0:128], tri
                                    )
                                exs[h] = ex
                            for i, h in enumerate(pair):
                                nc.tensor.matmul(
                                    dnt[32 * i : 32 * i + 1, jstart:512],
                                    ones,
                                    exs[h][:, :w],
                                    start=(kt == 0),
                                    stop=(kt == nkt - 1),
                                )
                            for h in pair:
                                nc.tensor.matmul(
                                    accs[h][:, jstart:512],
                                    vt[:, kt, :],
                                    exs[h][:, :w],
                                    start=(kt == 0),
                                    stop=(kt == nkt - 1),
                                )
                        # evacuate accumulators (DVE), normalize off-path
                        for i, h in enumerate(pair):
                            nc.vector.tensor_copy(at[h][:, qsl], accs[h])
                            dinv = dsm.tile([1, 512], f32, tag="dinv")
                            nc.vector.reciprocal(
                                dinv, dnt[32 * i : 32 * i + 1, :]
                            )
                            bc = scp.tile(
                                [128, 512], f32, tag="sc", name="bc"
                            )
                            nc.tensor.matmul(
                                bc, ones1f, dinv, start=True, stop=True
                            )
                            nc.vector.tensor_mul(
                                at[h][:, qsl], at[h][:, qsl], bc
                            )

            # ---------------- Phase 3: output projection ----------------
            with (
                tc.tile_pool(name=f"wosb{rep}", bufs=1) as wo_pool,
                tc.tile_pool(name=f"outp{rep}", bufs=8, space="PSUM") as outp,
                tc.tile_pool(name=f"stg{rep}", bufs=2) as stg,
            ):
                wo_sb = wo_pool.tile([128, nq, hid], bf16, tag="wosb")
                nc.sync.dma_start(
                    wo_sb, wo_d[:, :].rearrange("(t p) c -> p t c", p=128)
                )
                for st in range(STn):
                    ssl = slice(st * 128, (st + 1) * 128)
                    stg_t = stg.tile([128, hid], f16, tag="stg")
                    for g in range(2):
                        pos = [
                            outp.tile([128, 512], f32, tag="po", name="po")
                            for _ in range(HB // 2)
                        ]
                        for h in range(nq):
                            for nb in range(HB // 2):
                                nbi = g * (HB // 2) + nb
                                nc.tensor.matmul(
                                    pos[nb],
                                    at[h][:, ssl],
                                    wo_sb[:, h, nbi * 512 : (nbi + 1) * 512],
                                    start=(h == 0),
                                    stop=(h == nq - 1),
                                )
                        for nb in range(HB // 2):
                            nbi = g * (HB // 2) + nb
                            dstg = stg_t[:, nbi * 512 : (nbi + 1) * 512]
                            if nb % 2 == 0:
                                nc.vector.tensor_copy(dstg, pos[nb])
                            else:
                                nc.scalar.copy(dstg, pos[nb])
                    nc.sync.dma_start(out_d[ssl, :], stg_t)

    nc.compile()
    nc.finalize()
    return nc


def _prep_core_inputs(x_np, position_ids, Wq, Wk, Wv, Wo):
    """Host-side sharding/layout prep. Returns list of per-core input dicts."""
    scale = float(D) ** -0.5
    xT = np.ascontiguousarray(x_np.T).astype(BF)

    pos = np.asarray(position_ids).astype(np.float32)
    half = D // 2
    inv_freq = 1.0 / (ROPE_THETA ** (np.arange(half, dtype=np.float32) / half))
    ang = pos[:, None] * inv_freq[None, :]  # [S, 64]
    cosT = np.cos(ang).T.astype(np.float32)  # [64, S]
    sinT = np.sin(ang).T.astype(np.float32)
    cos2 = np.concatenate([cosT, cosT], axis=0)  # [128, S]
    sin2 = np.concatenate([-sinT, sinT], axis=0)  # [128, S]
    cos2 = np.ascontiguousarray(cos2)
    sin2 = np.ascontiguousarray(sin2)

    tri = np.triu(np.ones((128, 128), np.float32)).astype(BF)  # [k, q]: q >= k
    swpm = np.zeros((128, 128), np.float32)
    swpm[np.arange(128), (np.arange(128) + 64) % 128] = 1.0  # half-swap perm
    swpm = swpm.astype(BF)

    Wq_s = (np.asarray(Wq, np.float32) * scale)
    Wk = np.asarray(Wk, np.float32)
    Wv = np.asarray(Wv, np.float32)
    Wo = np.asarray(Wo, np.float32)

    in_maps = []
    for c in range(NCORES):
        qrows = Wq_s[c * NQ * D : (c + 1) * NQ * D]  # [512, HID]
        krows = Wk[c * D : (c + 1) * D]  # [128, HID]
        vrows = Wv[c * D : (c + 1) * D]
        # column order in wqkvT: k, v, q0..q3
        wqkv = np.concatenate([krows, vrows, qrows], axis=0)  # [768, HID]
        wqkvT = np.ascontiguousarray(wqkv.T).astype(BF)  # [HID, 768]
        woT = np.ascontiguousarray(Wo[:, c * NQ * D : (c + 1) * NQ * D].T).astype(
            BF
        )  # [512, HID]
        in_maps.append(
            {
                "xT": xT,
                "wqkvT": wqkvT,
                "woT": woT,
                "cos2": cos2,
                "sin2": sin2,
                "tri": tri,
                "swpm": swpm,
            }
        )
    return in_maps


def kernel(
    hidden_states,
    position_ids,
    page_indices,
    Wq,
    Wk,
    Wv,
    Wo,
    kv_cache,
    _trace=False,
):
    from concourse.bass_utils import run_bass_kernel_spmd

    x = np.asarray(hidden_states, np.float32)[0]  # [S, HID]
    pidx = np.asarray(page_indices)
    # write-then-gather through distinct pages is the identity
    assert len(np.unique(pidx)) == pidx.shape[0], "page_indices must be distinct"

    in_maps = _prep_core_inputs(x, position_ids, Wq, Wk, Wv, Wo)

    if "nc" not in _NC_CACHE:
        _NC_CACHE["nc"] = build_nc()
    nc = _NC_CACHE["nc"]

    res = run_bass_kernel_spmd(
        nc, in_maps, core_ids=list(range(NCORES)), trace=_trace
    )
    out = np.zeros((S, HID), np.float32)
    for c in range(NCORES):
        out += np.asarray(res.results[c]["out"], np.float32)
    if _trace:
        kernel.last_results = res
    return out[None].astype(np.float32)


# revision 4
# speedup vs baseline: 1.2786x; 1.2786x over previous
"""Trainium2 Bass kernel for Llama-style GQA attention prefill (S=2048), v6.

Sharding: tensor-parallel over heads across 8 NeuronCores.
Each core owns 4 query heads + 1 KV head (GQA group-aligned), computes
its partial o_proj contribution (Wo input-sharded), and the host sums
the 8 fp16 partials in fp32.

v6 changes vs v2:
- projections and attention are software-pipelined: attention for
  q-block qb is emitted immediately after projection seq-block sb=qb
  (causality makes its K/V available), so the Scalar-engine exp work of
  one q-block overlaps the PE projection matmuls of the next.
- PSUM is partitioned so both phases coexist: projection accumulators /
  transpose / RoPE-swap share one 3-slot pool; scores 2; attention
  accumulators 2; denominators 1 (pair shares a bank at rows 0/32).
- accumulator evacuation and the RoPE bf16 staging copy run on the
  Vector engine, keeping the Scalar engine for exp + fp32 staging.
- Wo is DMA'd late (after the x-tile pool frees) to fit SBUF.
- o_proj loops (seq-tile, hid-group, head); PSUM evacuation split
  between Scalar and Vector engines; fp16 output, one 1MB DMA per
  128-row seq tile.
"""

import sys

if "/opt/trn_rl_repo" not in sys.path:
    sys.path.insert(0, "/opt/trn_rl_repo")

import numpy as np
import ml_dtypes

BF = ml_dtypes.bfloat16

S = 2048
HID = 4096
D = 128
H = 32
HKV = 8
NCORES = 8
NQ = H // NCORES  # 4 query heads per core
ROPE_THETA = 10000.0

_NC_CACHE = {}


def build_nc(s=S, hid=HID, nq=NQ, reps=1):
    """Build the per-core Bass program (same program for all 8 cores).

    reps > 1 repeats the whole computation (for timing amplification:
    marginal cost per rep == true device time of one run).
    """
    import concourse.bass as bass
    import concourse.mybir as mybir
    import concourse.tile as tile
    from concourse import bacc
    from concourse.masks import make_identity

    f32 = mybir.dt.float32
    f16 = mybir.dt.float16
    bf16 = mybir.dt.bfloat16
    Exp = mybir.ActivationFunctionType.Exp

    KB = hid // 128   # hidden contraction blocks
    SBn = s // 512    # 512-wide sequence blocks
    KTn = s // 128    # 128-wide key tiles
    STn = s // 128    # 128-wide seq tiles
    NDB = nq + 2      # projection d-blocks: k, v, q0..q{nq-1}
    WC = NDB * 128    # wqkvT columns
    HB = hid // 512   # output hidden blocks

    nc = bacc.Bacc("TRN2")

    xT_d = nc.dram_tensor("xT", [hid, s], bf16, kind="ExternalInput")
    w_d = nc.dram_tensor("wqkvT", [hid, WC], bf16, kind="ExternalInput")
    wo_d = nc.dram_tensor("woT", [nq * 128, hid], bf16, kind="ExternalInput")
    cos2_d = nc.dram_tensor("cos2", [128, s], f32, kind="ExternalInput")
    sin2_d = nc.dram_tensor("sin2", [128, s], f32, kind="ExternalInput")
    tri_d = nc.dram_tensor("tri", [128, 128], bf16, kind="ExternalInput")
    swpm_d = nc.dram_tensor("swpm", [128, 128], bf16, kind="ExternalInput")
    out_d = nc.dram_tensor("out", [s, hid], f16, kind="ExternalOutput")

    with tile.TileContext(nc) as tc:
      for rep in range(reps):
        with (
            tc.tile_pool(name=f"const{rep}", bufs=1) as const_pool,
            tc.tile_pool(name=f"qkv{rep}", bufs=1) as qkv_pool,
        ):
            tri = const_pool.tile([128, 128], bf16, tag="tri")
            ones = const_pool.tile([128, 1], bf16, tag="ones")
            ones1f = const_pool.tile([1, 128], f32, tag="ones1f")
            ident = const_pool.tile([128, 128], bf16, tag="ident")
            swpm = const_pool.tile([128, 128], bf16, tag="swpm")
            nc.sync.dma_start(tri, tri_d[:, :])
            nc.sync.dma_start(swpm, swpm_d[:, :])
            nc.vector.memset(ones, 1.0)
            nc.vector.memset(ones1f, 1.0)
            make_identity(nc, ident)

            # persistent per-head tensors
            qk = [
                qkv_pool.tile([128, s], bf16, tag=f"qk{i}", name=f"qk{i}")
                for i in range(nq + 1)
            ]  # qk[0..nq-1] = q heads (T layout [d, s]); qk[nq] = kT
            vt = qkv_pool.tile([128, KTn, 128], bf16, tag="vt")
            at = [
                qkv_pool.tile([128, s], bf16, tag=f"at{i}", name=f"at{i}")
                for i in range(nq)
            ]  # attnT (unnormalized, then scaled in place) per head [d, s]
            kT = qk[nq]

            # ------- Phases 1+2 interleaved over seq/q blocks -------
            with (
                tc.tile_pool(name=f"csn{rep}", bufs=1) as cs_pool,
                tc.tile_pool(name=f"xt{rep}", bufs=2) as xt_pool,
                tc.tile_pool(name=f"wsb{rep}", bufs=1) as w_pool,
                tc.tile_pool(name=f"pp{rep}", bufs=3, space="PSUM") as pp,
                tc.tile_pool(name=f"scp{rep}", bufs=2, space="PSUM") as scp,
                tc.tile_pool(name=f"atp{rep}", bufs=1, space="PSUM") as atp,
                tc.tile_pool(name=f"dnp{rep}", bufs=1, space="PSUM") as dnp,
                tc.tile_pool(name=f"rtmp{rep}", bufs=3) as rt,
                tc.tile_pool(name=f"vstage{rep}", bufs=2) as vs,
                tc.tile_pool(name=f"exps{rep}", bufs=6) as exps,
                tc.tile_pool(name=f"dsm{rep}", bufs=4) as dsm,
            ):
                cos2 = cs_pool.tile([128, s], f32, tag="cos2")
                sin2 = cs_pool.tile([128, s], f32, tag="sin2")
                nc.sync.dma_start(cos2, cos2_d[:, :])
                nc.sync.dma_start(sin2, sin2_d[:, :])
                w_sb = w_pool.tile([128, KB, WC], bf16, tag="wsb")
                wv_view = w_d[:, :].rearrange("(t p) c -> p t c", p=128)
                # interleave weight and first-block x DMAs so the first
                # accumulation matmul unblocks after ~2 transfers
                xts0 = []
                for kb in range(KB):
                    nc.sync.dma_start(w_sb[:, kb, :], wv_view[:, kb, :])
                    xtile = xt_pool.tile(
                        [128, 512], bf16, tag=f"xt{kb}", name=f"xt{kb}"
                    )
                    nc.sync.dma_start(xtile, xT_d[kb * 128 : (kb + 1) * 128, 0:512])
                    xts0.append(xtile)

                for sb in range(SBn):
                    # ---- projections + RoPE for seq block sb ----
                    sl = slice(sb * 512, (sb + 1) * 512)
                    if sb == 0:
                        xts = xts0
                    else:
                        xts = []
                        for kb in range(KB):
                            xtile = xt_pool.tile(
                                [128, 512], bf16, tag=f"xt{kb}", name=f"xt{kb}"
                            )
                            nc.sync.dma_start(
                                xtile, xT_d[kb * 128 : (kb + 1) * 128, sl]
                            )
                            xts.append(xtile)
                    # d-block order: k(0), v(1), then q heads (2..)
                    for db in range(NDB):
                        ps = pp.tile([128, 512], f32, tag="pp", name="ps")
                        for kb in range(KB):
                            nc.tensor.matmul(
                                ps,
                                w_sb[:, kb, db * 128 : (db + 1) * 128],
                                xts[kb],
                                start=(kb == 0),
                                stop=(kb == KB - 1),
                            )
                        if db == 1:
                            # v: cast to bf16 then transpose to natural [s, d]
                            vstg = vs.tile([128, 512], bf16, tag="vstg")
                            nc.scalar.copy(vstg, ps)
                            for j in range(4):
                                tps = pp.tile(
                                    [128, 128], bf16, tag="pp", name="tps"
                                )
                                nc.tensor.transpose(
                                    tps, vstg[:, j * 128 : (j + 1) * 128], ident
                                )
                                nc.scalar.copy(vt[:, sb * 4 + j, :], tps)
                        else:
                            # RoPE: dst = p * COS2 + swap(p) * SIN2
                            # (half-swap via bf16 PE permutation matmul)
                            dst = qk[nq] if db == 0 else qk[db - 2]
                            pcp = rt.tile([128, 512], f32, tag="pcp")
                            nc.scalar.copy(pcp, ps)
                            pcb = vs.tile([128, 512], bf16, tag="pcb")
                            nc.vector.tensor_copy(pcb, ps)
                            sps = pp.tile(
                                [128, 512], f32, tag="pp", name="sps"
                            )
                            nc.tensor.matmul(
                                sps, swpm, pcb, start=True, stop=True
                            )
                            nc.vector.tensor_mul(pcp, pcp, cos2[:, sl])
                            swp = rt.tile([128, 512], f32, tag="swp")
                            nc.vector.tensor_mul(swp, sps, sin2[:, sl])
                            nc.vector.tensor_add(dst[:, sl], pcp, swp)

                    # ---- attention for q-block qb == sb ----
                    qb = sb
                    qsl = slice(qb * 512, (qb + 1) * 512)
                    nkt = 4 * qb + 4
                    for hp in range(0, nq, 2):  # head pairs
                        pair = (hp, hp + 1)
                        accs = {
                            h: atp.tile(
                                [128, 512], f32, tag=f"acc{h % 2}",
                                name=f"acc{h % 2}",
                            )
                            for h in pair
                        }
                        dnt = dnp.tile([128, 512], f32, tag="dn", name="dnt")
                        for kt in range(nkt):
                            jstart = max(0, 128 * (kt - 4 * qb))
                            w = 512 - jstart
                            q_lo = qb * 512 + jstart
                            exs = {}
                            for h in pair:
                                sc = scp.tile(
                                    [128, 512], f32, tag="sc", name="sc"
                                )
                                nc.tensor.matmul(
                                    sc[:, :w],
                                    kT[:, kt * 128 : (kt + 1) * 128],
                                    qk[h][:, q_lo : (qb + 1) * 512],
                                    start=True,
                                    stop=True,
                                )
                                ex = exps.tile(
                                    [128, 512], bf16, tag="ex", name="ex"
                                )
                                nc.scalar.activation(ex[:, :w], sc[:, :w], Exp)
                                if kt >= 4 * qb:
                                    nc.vector.tensor_mul(
                                        ex[:, 0:128], ex[:, 0:128], tri
                                    )
                                exs[h] = ex
                            for i, h in enumerate(pair):
                                nc.tensor.matmul(
                                    dnt[32 * i : 32 * i + 1, jstart:512],
                                    ones,
                                    exs[h][:, :w],
                                    start=(kt == 0),
                                    stop=(kt == nkt - 1),
                                )
                            for h in pair:
                                nc.tensor.matmul(
                                    accs[h][:, jstart:512],
                                    vt[:, kt, :],
                                    exs[h][:, :w],
                                    start=(kt == 0),
                                    stop=(kt == nkt - 1),
                                )
                        # evacuate accumulators (DVE), normalize off-path
                        for i, h in enumerate(pair):
                            nc.vector.tensor_copy(at[h][:, qsl], accs[h])
                            dinv = dsm.tile([1, 512], f32, tag="dinv")
                            nc.vector.reciprocal(
                                dinv, dnt[32 * i : 32 * i + 1, :]
                            )
                            bci = dsm.tile(
                                [128, 512], f32, tag="bci", name="bci"
                            )
                            nc.gpsimd.partition_broadcast(bci, dinv, 128)
                            nc.vector.tensor_mul(
                                at[h][:, qsl], at[h][:, qsl], bci
                            )

            # ---------------- Phase 3: output projection ----------------
            with (
                tc.tile_pool(name=f"wosb{rep}", bufs=1) as wo_pool,
                tc.tile_pool(name=f"outp{rep}", bufs=8, space="PSUM") as outp,
                tc.tile_pool(name=f"stg{rep}", bufs=2) as stg,
            ):
                wo_sb = wo_pool.tile([128, nq, hid], bf16, tag="wosb")
                nc.sync.dma_start(
                    wo_sb, wo_d[:, :].rearrange("(t p) c -> p t c", p=128)
                )
                for st in range(STn):
                    ssl = slice(st * 128, (st + 1) * 128)
                    stg_t = stg.tile([128, hid], f16, tag="stg")
                    for g in range(2):
                        pos = [
                            outp.tile([128, 512], f32, tag="po", name="po")
                            for _ in range(HB // 2)
                        ]
                        for h in range(nq):
                            for nb in range(HB // 2):
                                nbi = g * (HB // 2) + nb
                                nc.tensor.matmul(
                                    pos[nb],
                                    at[h][:, ssl],
                                    wo_sb[:, h, nbi * 512 : (nbi + 1) * 512],
                                    start=(h == 0),
                                    stop=(h == nq - 1),
                                )
                        for nb in range(HB // 2):
                            nbi = g * (HB // 2) + nb
                            dstg = stg_t[:, nbi * 512 : (nbi + 1) * 512]
                            if nb % 2 == 0:
                                nc.vector.tensor_copy(dstg, pos[nb])
                            else:
                                nc.scalar.copy(dstg, pos[nb])
                    nc.sync.dma_start(out_d[ssl, :], stg_t)

    nc.compile()
    nc.finalize()
    return nc


def _prep_core_inputs(x_np, position_ids, Wq, Wk, Wv, Wo):
    """Host-side sharding/layout prep. Returns list of per-core input dicts."""
    scale = float(D) ** -0.5
    xT = np.ascontiguousarray(x_np.T).astype(BF)

    pos = np.asarray(position_ids).astype(np.float32)
    half = D // 2
    inv_freq = 1.0 / (ROPE_THETA ** (np.arange(half, dtype=np.float32) / half))
    ang = pos[:, None] * inv_freq[None, :]  # [S, 64]
    cosT = np.cos(ang).T.astype(np.float32)  # [64, S]
    sinT = np.sin(ang).T.astype(np.float32)
    cos2 = np.concatenate([cosT, cosT], axis=0)  # [128, S]
    sin2 = np.concatenate([-sinT, sinT], axis=0)  # [128, S]
    cos2 = np.ascontiguousarray(cos2)
    sin2 = np.ascontiguousarray(sin2)

    tri = np.triu(np.ones((128, 128), np.float32)).astype(BF)  # [k, q]: q >= k
    swpm = np.zeros((128, 128), np.float32)
    swpm[np.arange(128), (np.arange(128) + 64) % 128] = 1.0  # half-swap perm
    swpm = swpm.astype(BF)

    Wq_s = (np.asarray(Wq, np.float32) * scale)
    Wk = np.asarray(Wk, np.float32)
    Wv = np.asarray(Wv, np.float32)
    Wo = np.asarray(Wo, np.float32)

    in_maps = []
    for c in range(NCORES):
        qrows = Wq_s[c * NQ * D : (c + 1) * NQ * D]  # [512, HID]
        krows = Wk[c * D : (c + 1) * D]  # [128, HID]
        vrows = Wv[c * D : (c + 1) * D]
        # column order in wqkvT: k, v, q0..q3
        wqkv = np.concatenate([krows, vrows, qrows], axis=0)  # [768, HID]
        wqkvT = np.ascontiguousarray(wqkv.T).astype(BF)  # [HID, 768]
        woT = np.ascontiguousarray(Wo[:, c * NQ * D : (c + 1) * NQ * D].T).astype(
            BF
        )  # [512, HID]
        in_maps.append(
            {
                "xT": xT,
                "wqkvT": wqkvT,
                "woT": woT,
                "cos2": cos2,
                "sin2": sin2,
                "tri": tri,
                "swpm": swpm,
            }
        )
    return in_maps


def kernel(
    hidden_states,
    position_ids,
    page_indices,
    Wq,
    Wk,
    Wv,
    Wo,
    kv_cache,
    _trace=False,
):
    from concourse.bass_utils import run_bass_kernel_spmd

    x = np.asarray(hidden_states, np.float32)[0]  # [S, HID]
    pidx = np.asarray(page_indices)
    # write-then-gather through distinct pages is the identity
    assert len(np.unique(pidx)) == pidx.shape[0], "page_indices must be distinct"

    in_maps = _prep_core_inputs(x, position_ids, Wq, Wk, Wv, Wo)

    if "nc" not in _NC_CACHE:
        _NC_CACHE["nc"] = build_nc()
    nc = _NC_CACHE["nc"]

    res = run_bass_kernel_spmd(
        nc, in_maps, core_ids=list(range(NCORES)), trace=_trace
    )
    out = np.zeros((S, HID), np.float32)
    for c in range(NCORES):
        out += np.asarray(res.results[c]["out"], np.float32)
    if _trace:
        kernel.last_results = res
    return out[None].astype(np.float32)
